# revision 1
# baseline (speedup 1.0000x reference)
"""BiDAF attention layer on 8 Trainium2 NeuronCores (Bass/Tile).

Math (per batch b):
  t[i,j]  = sum_d (c[i,d]*w_cq[d] + w_q[d]) * q[j,d]   (= cq + sq0[j])
  a       = softmax_j(t)            (biases b_c/b_q/b_cq cancel in softmax)
  c2q     = a @ q
  m[i]    = max_j t[i,j];  sc0[i] = c[i,:]@w_c
  bvec    = softmax_i(m + sc0)      (biases cancel here too)
  q2c     = bvec @ c
  out     = [c | c2q | c*c2q | c*q2c]

Sharding: data-parallel over batch, 4 batches per core, params replicated.

Implementation notes:
  - w_q is folded into the transposed-c operand: chatT = w_cq*cT + w_q,
    applied for free in the PSUM evacuation (tensor_scalar mult+add with
    per-partition vectors). The q@w_q row term then emerges from the score
    contraction itself -- no separate sq0 computation, no exp bias.
  - Score/attention matmuls run in fp16 (fp32 matmul is 2 passes + 2
    LDWEIGHTS on TRN2; 16-bit is 1 pass + FWL), accumulating f32 PSUM.
    c stays f32 end-to-end for the output blocks and products.
  - cT/qT built via PE transposes (contraction over d needs d on
    partitions for both operands). The c PSUM is evacuated twice: affine
    -> chatT (scores) and plain -> cT (for the sc0 matvec).
  - Scores computed twice on PE: once as t [i,j] (row-max for bvec), once
    as tT [j,i] so ScalarE exp() lands e^T in SBUF in exactly the lhsT
    layout the c2q matmul needs (no e-transposes).
  - Softmax skips max-subtraction (|t| <= ~10, exp safe in f32/fp16) and
    the row sum l is fused into the c2q matmul as a ones column of rhs.
  - DMA is split across the three DGE paths to avoid head-of-line
    blocking: c-in/c-out on sync(SP), stage-out on scalar(ACT), q-in
    (with f32->fp16 cast) and c4-out on gpsimd(SWDGE).
"""

import sys

if "/opt/trn_rl_repo" not in sys.path:
    sys.path.insert(0, "/opt/trn_rl_repo")

import numpy as np

import concourse.bass as bass
import concourse.tile as tile
from concourse import bacc, mybir
from concourse.bass import ds, ts
from concourse.masks import make_identity

B, CL, QL, D = 32, 1024, 512, 256
NCORES = 8
BS = B // NCORES  # batches per core
P = 128
F32 = mybir.dt.float32
F16 = mybir.dt.float16

NT = CL // P  # 8 i-tiles
NJ = QL // P  # 4 j-chunks
ND = D // P   # 2 d-chunks
NH = 2        # i-halves for the [j,i]-layout score matmul
IH = CL // NH  # 512
KPH = NT // NH  # i-tiles per half

Exp = mybir.ActivationFunctionType.Exp
AxX = mybir.AxisListType.X
Mult = mybir.AluOpType.mult
Add = mybir.AluOpType.add


def build_bass(bs: int = BS):
    nc = bacc.Bacc(None)
    c_d = nc.declare_dram_parameter("c", [bs, CL, D], F32, isOutput=False)
    q_d = nc.declare_dram_parameter("q", [bs, QL, D], F32, isOutput=False)
    wc_d = nc.declare_dram_parameter("wc_cols", [P, ND], F16, isOutput=False)
    wq_d = nc.declare_dram_parameter("wq_cols", [P, ND], F32, isOutput=False)
    wcq_d = nc.declare_dram_parameter("wcq_cols", [P, ND], F32, isOutput=False)
    out_d = nc.declare_dram_parameter("out", [bs, CL, 4 * D], F32, isOutput=True)

    with tile.TileContext(nc) as tc:
        with (
            tc.tile_pool(name="consts", bufs=1) as consts,
            tc.tile_pool(name="io", bufs=3) as io,
            tc.tile_pool(name="ins", bufs=3) as ins,
            tc.tile_pool(name="work", bufs=3) as work,
            tc.tile_pool(name="ps_t", bufs=2, space="PSUM") as ps_t,
            tc.tile_pool(name="ps_tT", bufs=1, space="PSUM") as ps_tT,
            tc.tile_pool(name="ps_s", bufs=2, space="PSUM") as ps_s,
        ):
            ident_f = consts.tile([P, P], F32)
            ident_h = consts.tile([P, P], F16)
            ones_f = consts.tile([P, P], F32)
            ones_h = consts.tile([1, QL], F16)
            neg_shift = consts.tile([P, 1], F32)
            wc_sb = consts.tile([P, ND], F16)
            wq_sb = consts.tile([P, ND], F32)
            wcq_sb = consts.tile([P, ND], F32)

            def emit_inputs(b):
                # q loaded once, cast f32 -> fp16 in-flight (SWDGE)
                q_sb = ins.tile([P, NJ, D + 1], F16, tag="q_sb")
                nc.gpsimd.dma_start(
                    out=q_sb[:, :, 0:D],
                    in_=q_d[b].rearrange("(t p) d -> p t d", p=P),
                )
                nc.vector.memset(q_sb[:, :, D : D + 1], 1.0)
                c_sb = ins.tile([P, NT, D], F32, tag="c_sb")
                nc.sync.dma_start(
                    out=c_sb, in_=c_d[b].rearrange("(t p) d -> p t d", p=P)
                )
                # fp16 copy of c (re-read + cast) for transposes and q2c
                c_h = ins.tile([P, NT, D], F16, tag="c_h")
                nc.gpsimd.dma_start(
                    out=c_h, in_=c_d[b].rearrange("(t p) d -> p t d", p=P)
                )
                # output block 0 is just c; store straight from SBUF (SP ring)
                ov = out_d[b].rearrange("(t p) x -> p t x", p=P)
                nc.sync.dma_start(out=ov[:, :, 0:D], in_=c_sb)
                return c_sb, q_sb, c_h, ov

            pending = [emit_inputs(0)]

            for b in range(bs):
                c_sb, q_sb, c_h, ov = pending.pop(0)

                if b == 0:
                    nc.sync.dma_start(out=wc_sb, in_=wc_d[:])
                    nc.sync.dma_start(out=wq_sb, in_=wq_d[:])
                    nc.sync.dma_start(out=wcq_sb, in_=wcq_d[:])
                    make_identity(nc, ident_h)
                    make_identity(nc, ident_f)
                    nc.vector.memset(ones_f, 1.0)
                    nc.vector.memset(ones_h, 1.0)
                    nc.vector.memset(neg_shift, -2.5)
                else:
                    pass
                # prefetch up to two batches ahead of this batch's gpsimd
                # work so loads aren't FIFO-blocked behind c4 products
                if b == 0:
                    for nb in (1, 2):
                        if nb < bs:
                            pending.append(emit_inputs(nb))
                elif b + 2 < bs:
                    pending.append(emit_inputs(b + 2))

                # ------------- transpose q -> qT (fp16) -------------
                qT = work.tile([P, ND, QL], F16, tag="qT")
                for dc in range(ND):
                    pst = ps_t.tile([P, QL], F16, tag="t")
                    for jc in range(NJ):
                        nc.tensor.transpose(
                            pst[:, ts(jc, P)], q_sb[:, jc, ts(dc, P)], ident_h
                        )
                    if dc == 0:
                        nc.scalar.copy(qT[:, dc], pst)
                    else:
                        nc.vector.tensor_copy(qT[:, dc], pst)

                # ---- transpose c_h -> cT (plain) + chatT (affine) ----
                cT = work.tile([P, ND, CL], F16, tag="cT")
                chatT = work.tile([P, ND, CL], F16, tag="chatT")
                for dc in range(ND):
                    for h in range(NH):
                        pst = ps_t.tile([P, IH], F16, tag="t")
                        for k in range(KPH):
                            it = h * KPH + k
                            nc.tensor.transpose(
                                pst[:, ts(k, P)], c_h[:, it, ts(dc, P)], ident_h
                            )
                        sl = ds(h * IH, IH)
                        nc.vector.tensor_scalar(
                            out=chatT[:, dc, sl],
                            in0=pst,
                            scalar1=wcq_sb[:, dc : dc + 1],
                            scalar2=wq_sb[:, dc : dc + 1],
                            op0=Mult,
                            op1=Add,
                        )
                        nc.scalar.copy(cT[:, dc, sl], pst)

                # ---- sc0 rows: [1, IH] per half via M=1 matmuls (fp16) ----
                sc0_row = work.tile([1, CL], F16, tag="sc0r")
                for h in range(NH):
                    ps_sr = ps_t.tile([1, IH], F32, tag="t")
                    for dc in range(ND):
                        nc.tensor.matmul(
                            ps_sr,
                            wc_sb[:, dc : dc + 1],
                            cT[:, dc, ds(h * IH, IH)],
                            start=(dc == 0),
                            stop=(dc == ND - 1),
                        )
                    if h == 0:
                        nc.scalar.copy(sc0_row[0:1, ds(h * IH, IH)], ps_sr)
                    else:
                        nc.vector.tensor_copy(sc0_row[0:1, ds(h * IH, IH)], ps_sr)

                # ---- phase M: scores, e^T, and row maxes ----
                m_all = work.tile([P, NT], F32, tag="m_all")
                eTs = []
                for h in range(NH):
                    tTq = ps_tT.tile([P, NJ, IH], F32, tag="tTq")
                    for jc in range(NJ):
                        for dc in range(ND):
                            nc.tensor.matmul(
                                tTq[:, jc],
                                qT[:, dc, ts(jc, P)],
                                chatT[:, dc, ds(h * IH, IH)],
                                start=(dc == 0),
                                stop=(dc == ND - 1),
                            )
                    eT = work.tile([P, NJ, IH], F16, tag="eT")
                    eTs.append(eT)
                    for jc in range(NJ):
                        nc.scalar.activation(eT[:, jc], tTq[:, jc], Exp)

                    for k in range(KPH):
                        it = h * KPH + k
                        pt = ps_t.tile([P, QL], F32, tag="t")
                        for dc in range(ND):
                            nc.tensor.matmul(
                                pt,
                                chatT[:, dc, ts(it, P)],
                                qT[:, dc],
                                start=(dc == 0),
                                stop=False,
                            )
                        # + sc0[i] broadcast over j (K=1): m_all = max_j t + sc0
                        nc.tensor.matmul(
                            pt,
                            sc0_row[0:1, ts(it, P)],
                            ones_h,
                            start=False,
                            stop=True,
                        )
                        nc.vector.reduce_max(m_all[:, it : it + 1], pt, AxX)

                # ---- bvec numerators (ebv in fp16, shifted by -2.5) ----
                ebv_h = work.tile([P, NT], F16, tag="ebvh")
                nc.scalar.activation(ebv_h, m_all, Exp, bias=neg_shift[:, 0:1])
                colsum = work.tile([P, 1], F32, tag="colsum")
                nc.vector.reduce_sum(colsum, ebv_h, AxX)

                # ---- phase 2a: c2q matmuls for first half ----
                stage = io.tile([P, NT, 2 * D], F32, tag="stage")

                def mm2_tile(h, k):
                    it = h * KPH + k
                    po = ps_s.tile([P, D + 1], F32, tag="s")
                    for jc in range(NJ):
                        nc.tensor.matmul(
                            po,
                            eTs[h][:, jc, ts(k, P)],
                            q_sb[:, jc],
                            start=(jc == 0),
                            stop=(jc == NJ - 1),
                        )
                    linv = work.tile([P, 1], F32, tag="linv")
                    nc.vector.reciprocal(linv, po[:, D : D + 1])
                    nc.scalar.mul(stage[:, it, 0:D], po[:, 0:D], linv)
                    nc.vector.tensor_mul(
                        stage[:, it, D : 2 * D],
                        c_sb[:, it],
                        stage[:, it, 0:D],
                    )

                for k in range(KPH):
                    mm2_tile(0, k)
                nc.scalar.dma_start(
                    out=ov[:, 0:KPH, D : 3 * D], in_=stage[:, 0:KPH]
                )

                # ---- q2c chain (PE parts emitted after mm2 h0 so the
                # colsum/ebv dependencies are already satisfied) ----
                ps_tot = ps_s.tile([P, 1], F32, tag="s")
                nc.tensor.matmul(ps_tot, ones_f, colsum, start=True, stop=True)
                totinv = work.tile([P, 1], F32, tag="totinv")
                nc.vector.reciprocal(totinv, ps_tot)
                ps_q2c = ps_s.tile([1, D], F32, tag="s")
                for it in range(NT):
                    nc.tensor.matmul(
                        ps_q2c,
                        ebv_h[:, it : it + 1],
                        c_h[:, it],
                        start=(it == 0),
                        stop=(it == NT - 1),
                    )
                q2c_row = work.tile([1, D], F32, tag="q2cr")
                nc.vector.tensor_scalar_mul(q2c_row, ps_q2c, totinv[0:1, 0:1])
                ps_q2cb = ps_t.tile([P, D], F32, tag="t")
                nc.tensor.matmul(
                    ps_q2cb, ones_f[0:1, :], q2c_row, start=True, stop=True
                )
                q2c_sb = work.tile([P, D], F32, tag="q2csb")
                nc.scalar.copy(q2c_sb, ps_q2cb)
                c4st = io.tile([P, NT, D], F32, tag="c4st")
                for it in range(NT):
                    nc.gpsimd.tensor_mul(c4st[:, it], c_sb[:, it], q2c_sb)
                nc.gpsimd.dma_start(out=ov[:, :, 3 * D : 4 * D], in_=c4st)

                # ---- phase 2b: second half ----
                for k in range(KPH):
                    mm2_tile(1, k)
                nc.scalar.dma_start(
                    out=ov[:, KPH:NT, D : 3 * D], in_=stage[:, KPH:NT]
                )

    nc.compile()
    return nc


_NC_CACHE = {}


def _get_nc(bs: int = BS):
    if bs not in _NC_CACHE:
        _NC_CACHE[bs] = build_bass(bs)
    return _NC_CACHE[bs]


def _param_maps(w_c, w_q, w_cq):
    wc_cols = np.ascontiguousarray(
        np.asarray(w_c, np.float32).reshape(ND, P).T.astype(np.float16)
    )
    wq_cols = np.ascontiguousarray(np.asarray(w_q, np.float32).reshape(ND, P).T)
    wcq_cols = np.ascontiguousarray(
        np.asarray(w_cq, np.float32).reshape(ND, P).T
    )
    return wc_cols, wq_cols, wcq_cols


def _run(c, q, w_c, w_q, w_cq, trace=False, **trace_kwargs):
    from concourse.bass_utils import run_bass_kernel_spmd

    c = np.asarray(c, np.float32)
    q = np.asarray(q, np.float32)
    wc_cols, wq_cols, wcq_cols = _param_maps(w_c, w_q, w_cq)

    nc = _get_nc(BS)
    in_maps = []
    for k in range(NCORES):
        in_maps.append(
            {
                "c": np.ascontiguousarray(c[k * BS : (k + 1) * BS]),
                "q": np.ascontiguousarray(q[k * BS : (k + 1) * BS]),
                "wc_cols": wc_cols,
                "wq_cols": wq_cols,
                "wcq_cols": wcq_cols,
            }
        )
    res = None
    last_err = None
    for attempt in range(3):
        try:
            res = run_bass_kernel_spmd(
                nc,
                in_maps,
                core_ids=list(range(NCORES)),
                trace=trace,
                **trace_kwargs,
            )
            break
        except Exception as e:  # transient device wedges clear on retry
            last_err = e
            if "UNRECOVERABLE" not in str(e) and "UNAVAILABLE" not in str(e):
                raise
    if res is None:
        raise last_err
    out = np.concatenate([res.results[k]["out"] for k in range(NCORES)], axis=0)
    return out, res


def kernel(c, q, w_c, b_c, w_q, b_q, w_cq, b_cq):
    # b_c/b_q/b_cq provably cancel in both softmaxes; output doesn't use them.
    out, _ = _run(c, q, w_c, w_q, w_cq)
    return out



# revision 7
# speedup vs baseline: 1.2151x; 1.2151x over previous
"""BiDAF attention layer on 8 Trainium2 NeuronCores (Bass/Tile), v2.

Math (per batch b):
  t[i,j]  = sum_d (c[i,d]*w_cq[d] + w_q[d]) * q[j,d]   (= cq + sq0[j])
  a       = softmax_j(t)            (biases b_c/b_q/b_cq cancel in softmax)
  c2q     = a @ q
  m[i]    = max_j t[i,j];  sc0[i] = c[i,:]@w_c
  bvec    = softmax_i(m + sc0)      (biases cancel here too)
  q2c     = bvec @ c
  out     = [c | c2q | c*c2q | c*q2c]

Sharding: data-parallel over batch, 4 batches per core, params replicated.

v2 changes vs the 136us baseline (all fp16-matmul, PE-bound):
  - Scores computed ONCE, only in the transposed [j,i] layout the c2q
    matmul needs. The row max over j (a partition-dim reduction there) is
    done as max_j t = log max_j e^t: chunk-max of e^T on DVE, 8 small PE
    transposes of the [j,512] partials, then a free-dim reduce_max. This
    kills the entire second score pass (24 N=512 matmuls per batch).
  - sc0 is computed directly in column layout by 16 tiny N=1 matmuls
    (lhsT = cT tile, rhs = w_c column), then folded into the bvec
    numerators as ebv = (max_j e^t) * exp(sc0 - 2.5).
  - c is loaded once, cast f32->fp16 in flight (SWDGE); no separate f32
    copy.  All PE work is fp16 (1 cycle/row).
  - The output is written as fp16 (tolerance is 2e-2; fp16 rounding is
    ~2e-4) and upcast to f32 on the host: halves the store traffic.
  - Output staged as one [P, NT, 3D] tile (c2q | c*c2q | c*q2c) so each
    half is a single store; block0 (= c) stores straight from c_sb.
  - DMA paths: loads on SWDGE (casts), block0 on the scalar HWDGE ring,
    stage stores on the sync ring (avoids head-of-line blocking between
    prefetched block0 stores and current-batch stage stores).
  - PE emission order interleaves transposes / score matmuls / c2q
    matmuls / q2c chain so the PE never idles waiting on ACT exp or DVE
    evacuations (idle gaps also drop the PE out of its 2.4GHz p-state).
"""

import sys

if "/opt/trn_rl_repo" not in sys.path:
    sys.path.insert(0, "/opt/trn_rl_repo")

import numpy as np

import concourse.bass as bass
import concourse.tile as tile
from concourse import bacc, mybir
from concourse.bass import ds, ts
from concourse.masks import make_identity

B, CL, QL, D = 32, 1024, 512, 256
NCORES = 8
BS = B // NCORES  # batches per core
P = 128
F32 = mybir.dt.float32
F16 = mybir.dt.float16

NT = CL // P  # 8 i-tiles
NJ = QL // P  # 4 j-chunks
ND = D // P   # 2 d-chunks
NH = 2        # i-halves for the [j,i]-layout score matmul
IH = CL // NH  # 512
KPH = NT // NH  # i-tiles per half

Exp = mybir.ActivationFunctionType.Exp
AxX = mybir.AxisListType.X
Mult = mybir.AluOpType.mult
Add = mybir.AluOpType.add


def build_bass(bs: int = BS):
    nc = bacc.Bacc(None)
    c_d = nc.declare_dram_parameter("c", [bs, CL, D], F32, isOutput=False)
    q_d = nc.declare_dram_parameter("q", [bs, QL, D], F32, isOutput=False)
    wc_d = nc.declare_dram_parameter("wc_cols", [P, ND], F16, isOutput=False)
    wq_d = nc.declare_dram_parameter("wq_cols", [P, ND], F32, isOutput=False)
    wcq_d = nc.declare_dram_parameter("wcq_cols", [P, ND], F32, isOutput=False)
    out_d = nc.declare_dram_parameter("out", [bs, CL, 4 * D], F16, isOutput=True)

    D2, D3 = 2 * D, 3 * D

    with tile.TileContext(nc) as tc:
        with (
            tc.tile_pool(name="consts", bufs=1) as consts,
            tc.tile_pool(name="ins", bufs=3) as ins,
            tc.tile_pool(name="work", bufs=2) as work,
            tc.tile_pool(name="stg", bufs=2) as stg,
            tc.tile_pool(name="ps_mm", bufs=3, space="PSUM") as ps_mm,
            tc.tile_pool(name="ps_tr", bufs=2, space="PSUM") as ps_tr,
            tc.tile_pool(name="ps_s", bufs=2, space="PSUM") as ps_s,
            tc.tile_pool(name="ps_q", bufs=1, space="PSUM") as ps_q,
        ):
            ident_h = consts.tile([P, P], F16)
            ones_f = consts.tile([P, P], F32)
            ones_h1 = consts.tile([1, P], F16)
            wc_sb = consts.tile([P, ND], F16)
            wq_sb = consts.tile([P, ND], F32)
            wcq_sb = consts.tile([P, ND], F32)
            neg_shift = consts.tile([P, 1], F32)

            # params on the sync ring, identity/memsets on DVE+gpsimd --
            # all overlap the first SWDGE loads
            nc.sync.dma_start(out=wc_sb, in_=wc_d[:])
            nc.sync.dma_start(out=wq_sb, in_=wq_d[:])
            nc.sync.dma_start(out=wcq_sb, in_=wcq_d[:])

            def emit_inputs(b, chunked):
                q_sb = ins.tile([P, NJ, D + 1], F16, tag="q_sb")
                qv = q_d[b].rearrange("(t p) d -> p t d", p=P)
                if chunked:
                    nc.gpsimd.dma_start(out=q_sb[:, 0:2, 0:D], in_=qv[:, 0:2])
                    nc.gpsimd.dma_start(out=q_sb[:, 2:4, 0:D], in_=qv[:, 2:4])
                else:
                    nc.gpsimd.dma_start(out=q_sb[:, :, 0:D], in_=qv)
                nc.vector.memset(q_sb[:, :, D : D + 1], 1.0)
                c_sb = ins.tile([P, NT, D], F16, tag="c_sb")
                cv = c_d[b].rearrange("(t p) d -> p t d", p=P)
                if chunked:
                    nc.gpsimd.dma_start(out=c_sb[:, 0:KPH], in_=cv[:, 0:KPH])
                    nc.gpsimd.dma_start(out=c_sb[:, KPH:NT], in_=cv[:, KPH:NT])
                else:
                    nc.gpsimd.dma_start(out=c_sb, in_=cv)
                ov = out_d[b].rearrange("(t p) x -> p t x", p=P)
                # output block 0 is just c (fp16); scalar HWDGE ring
                nc.scalar.dma_start(out=ov[:, :, 0:D], in_=c_sb)
                return c_sb, q_sb, ov

            pending = [emit_inputs(0, True)]
            make_identity(nc, ident_h)
            nc.vector.memset(ones_f, 1.0)
            nc.vector.memset(ones_h1, 1.0)
            nc.vector.memset(neg_shift, -2.5)

            for b in range(bs):
                c_sb, q_sb, ov = pending.pop(0)
                # prefetch up to two batches ahead
                if b == 0:
                    for nb in (1, 2):
                        if nb < bs:
                            pending.append(emit_inputs(nb, False))
                elif b + 2 < bs:
                    pending.append(emit_inputs(b + 2, False))

                # ---------------- transposes (PE, fp16) ----------------
                qT = work.tile([P, ND, QL], F16, tag="qT")
                for dc in range(ND):
                    pst = ps_tr.tile([P, QL], F16, tag="tr")
                    for jc in range(NJ):
                        nc.tensor.transpose(
                            pst[:, ts(jc, P)], q_sb[:, jc, ts(dc, P)], ident_h
                        )
                    if dc == 0:
                        nc.scalar.copy(qT[:, dc], pst)
                    else:
                        nc.vector.tensor_copy(qT[:, dc], pst)

                cT = work.tile([P, ND, CL], F16, tag="cT")
                chatT = work.tile([P, ND, CL], F16, tag="chatT")
                for h in range(NH):
                    for dc in range(ND):
                        pst = ps_tr.tile([P, IH], F16, tag="tr")
                        for k in range(KPH):
                            nc.tensor.transpose(
                                pst[:, ts(k, P)],
                                c_sb[:, h * KPH + k, ts(dc, P)],
                                ident_h,
                            )
                        sl = ds(h * IH, IH)
                        nc.vector.tensor_scalar(
                            out=chatT[:, dc, sl],
                            in0=pst,
                            scalar1=wcq_sb[:, dc : dc + 1],
                            scalar2=wq_sb[:, dc : dc + 1],
                            op0=Mult,
                            op1=Add,
                        )
                        nc.scalar.copy(cT[:, dc, sl], pst)

                # ---------------- scores h0: tT + exp ----------------
                eT0 = work.tile([P, NJ, IH], F16, tag="eT0")
                eT1 = work.tile([P, NJ, IH], F16, tag="eT1")
                eT = [eT0, eT1]

                def score_chunk(h, jc):
                    pmm = ps_mm.tile([P, IH], F32, tag="tT")
                    for dc in range(ND):
                        nc.tensor.matmul(
                            pmm,
                            qT[:, dc, ts(jc, P)],
                            chatT[:, dc, ds(h * IH, IH)],
                            start=(dc == 0),
                            stop=(dc == ND - 1),
                        )
                    nc.scalar.activation(eT[h][:, jc], pmm, Exp)

                for jc in range(NJ):
                    score_chunk(0, jc)

                # ---- sc0 in column layout: 16 tiny N=1 matmuls ----
                psc = ps_tr.tile([P, NT], F32, tag="tr")
                for it in range(NT):
                    for dc in range(ND):
                        nc.tensor.matmul(
                            psc[:, it : it + 1],
                            cT[:, dc, ts(it, P)],
                            wc_sb[:, dc : dc + 1],
                            start=(dc == 0),
                            stop=(dc == ND - 1),
                        )
                e_sc0 = work.tile([P, NT], F16, tag="esc0")
                nc.scalar.activation(e_sc0, psc, Exp, bias=neg_shift[:, 0:1])

                # ---------------- scores h1 + chunk-max h0 ----------------
                for jc in range(NJ):
                    score_chunk(1, jc)
                    if jc == 1:
                        # row max over j, stage 1: max across j-chunks (DVE)
                        M1a0 = work.tile([P, 2, IH], F16, tag="m1a0")
                        nc.vector.tensor_max(
                            M1a0, eT[0][:, 0:2, :], eT[0][:, 2:4, :]
                        )
                        M1h0 = work.tile([P, IH], F16, tag="m1h0")
                        nc.vector.tensor_max(M1h0, M1a0[:, 0, :], M1a0[:, 1, :])

                # ---------------- c2q h0 + row-max transposes ----------------
                stage = stg.tile([P, NT, D3], F16, tag="stage")
                linv = work.tile([P, NT], F32, tag="linv")
                Me16 = work.tile([P, NT], F16, tag="me")

                def mm2_tile(h, k):
                    it = h * KPH + k
                    po = ps_s.tile([P, D + 1], F32, tag="po")
                    for jc in range(NJ):
                        nc.tensor.matmul(
                            po,
                            eT[h][:, jc, ts(k, P)],
                            q_sb[:, jc],
                            start=(jc == 0),
                            stop=(jc == NJ - 1),
                        )
                    nc.vector.reciprocal(linv[:, it : it + 1], po[:, D : D + 1])
                    nc.scalar.mul(
                        stage[:, it, 0:D], po[:, 0:D], linv[:, it : it + 1]
                    )

                def m1t(h, m1h_tile):
                    # stage 2: transpose the [j,512] partial maxes, then a
                    # free-dim reduce gives m in column layout
                    ptm = ps_tr.tile([P, KPH, P], F16, tag="tr")
                    for k in range(KPH):
                        nc.tensor.transpose(
                            ptm[:, k, :], m1h_tile[:, ts(k, P)], ident_h
                        )
                    nc.vector.reduce_max(
                        Me16[:, h * KPH : (h + 1) * KPH], ptm, AxX
                    )

                mm2_tile(0, 0)
                m1t(0, M1h0)
                mm2_tile(0, 1)
                # chunk-max h1 (eT[1] complete by now)
                M1a1 = work.tile([P, 2, IH], F16, tag="m1a1")
                nc.vector.tensor_max(M1a1, eT[1][:, 0:2, :], eT[1][:, 2:4, :])
                M1h1 = work.tile([P, IH], F16, tag="m1h1")
                nc.vector.tensor_max(M1h1, M1a1[:, 0, :], M1a1[:, 1, :])
                mm2_tile(0, 2)
                m1t(1, M1h1)
                # bvec numerators: ebv = (max_j e^t) * e^(sc0-2.5)
                ebv = work.tile([P, NT], F16, tag="ebv")
                nc.vector.tensor_mul(ebv, Me16, e_sc0)
                colsum = work.tile([P, 1], F32, tag="colsum")
                nc.vector.reduce_sum(colsum, ebv, AxX)
                mm2_tile(0, 3)
                # c*c2q for h0 (one strided DVE op over 4 tiles)
                nc.vector.tensor_mul(
                    stage[:, 0:KPH, D:D2],
                    c_sb[:, 0:KPH],
                    stage[:, 0:KPH, 0:D],
                )

                # ---------------- q2c chain + c2q h1 ----------------
                ps_tot = ps_q.tile([P, 1], F32, tag="q")
                nc.tensor.matmul(ps_tot, ones_f, colsum, start=True, stop=True)
                totinv = work.tile([P, 1], F32, tag="totinv")
                nc.vector.reciprocal(totinv, ps_tot)
                mm2_tile(1, 0)
                ps_q2c = ps_q.tile([1, D], F32, tag="q")
                for it in range(NT):
                    nc.tensor.matmul(
                        ps_q2c,
                        ebv[:, it : it + 1],
                        c_sb[:, it],
                        start=(it == 0),
                        stop=(it == NT - 1),
                    )
                q2c_row = work.tile([1, D], F16, tag="q2cr")
                nc.vector.tensor_scalar_mul(q2c_row, ps_q2c, totinv[0:1, 0:1])
                mm2_tile(1, 1)
                ps_q2cb = ps_q.tile([P, D], F32, tag="q")
                nc.tensor.matmul(
                    ps_q2cb, ones_h1, q2c_row, start=True, stop=True
                )
                q2c_sb = work.tile([P, D], F16, tag="q2csb")
                nc.scalar.copy(q2c_sb, ps_q2cb)
                # c*q2c for h0 tiles (gpsimd), then store the h0 half
                for it in range(KPH):
                    nc.gpsimd.tensor_mul(
                        stage[:, it, D2:D3], c_sb[:, it], q2c_sb
                    )
                nc.sync.dma_start(
                    out=ov[:, 0:KPH, D : 4 * D], in_=stage[:, 0:KPH]
                )
                mm2_tile(1, 2)
                mm2_tile(1, 3)
                nc.vector.tensor_mul(
                    stage[:, KPH:NT, D:D2],
                    c_sb[:, KPH:NT],
                    stage[:, KPH:NT, 0:D],
                )
                for it in range(KPH, NT):
                    nc.gpsimd.tensor_mul(
                        stage[:, it, D2:D3], c_sb[:, it], q2c_sb
                    )
                nc.sync.dma_start(
                    out=ov[:, KPH:NT, D : 4 * D], in_=stage[:, KPH:NT]
                )

    nc.compile()
    return nc


_NC_CACHE = {}


def _get_nc(bs: int = BS):
    if bs not in _NC_CACHE:
        _NC_CACHE[bs] = build_bass(bs)
    return _NC_CACHE[bs]


def _param_maps(w_c, w_q, w_cq):
    wc_cols = np.ascontiguousarray(
        np.asarray(w_c, np.float32).reshape(ND, P).T.astype(np.float16)
    )
    wq_cols = np.ascontiguousarray(np.asarray(w_q, np.float32).reshape(ND, P).T)
    wcq_cols = np.ascontiguousarray(
        np.asarray(w_cq, np.float32).reshape(ND, P).T
    )
    return wc_cols, wq_cols, wcq_cols


def _run(c, q, w_c, w_q, w_cq, trace=False, **trace_kwargs):
    from concourse.bass_utils import run_bass_kernel_spmd

    c = np.asarray(c, np.float32)
    q = np.asarray(q, np.float32)
    wc_cols, wq_cols, wcq_cols = _param_maps(w_c, w_q, w_cq)

    nc = _get_nc(BS)
    in_maps = []
    for k in range(NCORES):
        in_maps.append(
            {
                "c": np.ascontiguousarray(c[k * BS : (k + 1) * BS]),
                "q": np.ascontiguousarray(q[k * BS : (k + 1) * BS]),
                "wc_cols": wc_cols,
                "wq_cols": wq_cols,
                "wcq_cols": wcq_cols,
            }
        )
    res = None
    last_err = None
    for attempt in range(3):
        try:
            res = run_bass_kernel_spmd(
                nc,
                in_maps,
                core_ids=list(range(NCORES)),
                trace=trace,
                **trace_kwargs,
            )
            break
        except Exception as e:  # transient device wedges clear on retry
            last_err = e
            if "UNRECOVERABLE" not in str(e) and "UNAVAILABLE" not in str(e):
                raise
    if res is None:
        raise last_err
    out = np.concatenate([res.results[k]["out"] for k in range(NCORES)], axis=0)
    return np.ascontiguousarray(out.astype(np.float32)), res


def kernel(c, q, w_c, b_c, w_q, b_q, w_cq, b_cq):
    # b_c/b_q/b_cq provably cancel in both softmaxes; output doesn't use them.
    out, _ = _run(c, q, w_c, w_q, w_cq)
    return out


# revision 9
# speedup vs baseline: 1.2838x; 1.0565x over previous
"""BiDAF attention layer on 8 Trainium2 NeuronCores (Bass/Tile), v2.

Math (per batch b):
  t[i,j]  = sum_d (c[i,d]*w_cq[d] + w_q[d]) * q[j,d]   (= cq + sq0[j])
  a       = softmax_j(t)            (biases b_c/b_q/b_cq cancel in softmax)
  c2q     = a @ q
  m[i]    = max_j t[i,j];  sc0[i] = c[i,:]@w_c
  bvec    = softmax_i(m + sc0)      (biases cancel here too)
  q2c     = bvec @ c
  out     = [c | c2q | c*c2q | c*q2c]

Sharding: data-parallel over batch, 4 batches per core, params replicated.

v2 changes vs the 136us baseline (all fp16-matmul, PE-bound):
  - Scores computed ONCE, only in the transposed [j,i] layout the c2q
    matmul needs. The row max over j (a partition-dim reduction there) is
    done as max_j t = log max_j e^t: chunk-max of e^T on DVE, 8 small PE
    transposes of the [j,512] partials, then a free-dim reduce_max. This
    kills the entire second score pass (24 N=512 matmuls per batch).
  - sc0 is computed directly in column layout by 16 tiny N=1 matmuls
    (lhsT = cT tile, rhs = w_c column), then folded into the bvec
    numerators as ebv = (max_j e^t) * exp(sc0 - 2.5).
  - c is loaded once, cast f32->fp16 in flight (SWDGE); no separate f32
    copy.  All PE work is fp16 (1 cycle/row).
  - The output is written as fp16 (tolerance is 2e-2; fp16 rounding is
    ~2e-4) and upcast to f32 on the host: halves the store traffic.
  - Output staged as one [P, NT, 3D] tile (c2q | c*c2q | c*q2c) so each
    half is a single store; block0 (= c) stores straight from c_sb.
  - DMA paths: loads on SWDGE (casts), block0 on the scalar HWDGE ring,
    stage stores on the sync ring (avoids head-of-line blocking between
    prefetched block0 stores and current-batch stage stores).
  - PE emission order interleaves transposes / score matmuls / c2q
    matmuls / q2c chain so the PE never idles waiting on ACT exp or DVE
    evacuations (idle gaps also drop the PE out of its 2.4GHz p-state).
"""

import sys

if "/opt/trn_rl_repo" not in sys.path:
    sys.path.insert(0, "/opt/trn_rl_repo")

import numpy as np

import concourse.bass as bass
import concourse.tile as tile
from concourse import bacc, mybir
from concourse.bass import ds, ts
from concourse.masks import make_identity

B, CL, QL, D = 32, 1024, 512, 256
NCORES = 8
BS = B // NCORES  # batches per core
P = 128
F32 = mybir.dt.float32
F16 = mybir.dt.float16

NT = CL // P  # 8 i-tiles
NJ = QL // P  # 4 j-chunks
ND = D // P   # 2 d-chunks
NH = 2        # i-halves for the [j,i]-layout score matmul
IH = CL // NH  # 512
KPH = NT // NH  # i-tiles per half

Exp = mybir.ActivationFunctionType.Exp
AxX = mybir.AxisListType.X
Mult = mybir.AluOpType.mult
Add = mybir.AluOpType.add


def build_bass(bs: int = BS):
    nc = bacc.Bacc(None)
    c_d = nc.declare_dram_parameter("c", [bs, CL, D], F32, isOutput=False)
    q_d = nc.declare_dram_parameter("q", [bs, QL, D], F32, isOutput=False)
    wc_d = nc.declare_dram_parameter("wc_cols", [P, ND], F16, isOutput=False)
    wq_d = nc.declare_dram_parameter("wq_cols", [P, ND], F32, isOutput=False)
    wcq_d = nc.declare_dram_parameter("wcq_cols", [P, ND], F32, isOutput=False)
    out_d = nc.declare_dram_parameter("out", [bs, CL, 4 * D], F16, isOutput=True)

    D2, D3 = 2 * D, 3 * D

    with tile.TileContext(nc) as tc:
        with (
            tc.tile_pool(name="consts", bufs=1) as consts,
            tc.tile_pool(name="ins", bufs=3) as ins,
            tc.tile_pool(name="work", bufs=2) as work,
            tc.tile_pool(name="stg", bufs=2) as stg,
            tc.tile_pool(name="ps_mm", bufs=3, space="PSUM") as ps_mm,
            tc.tile_pool(name="ps_tr", bufs=2, space="PSUM") as ps_tr,
            tc.tile_pool(name="ps_s", bufs=2, space="PSUM") as ps_s,
            tc.tile_pool(name="ps_q", bufs=1, space="PSUM") as ps_q,
        ):
            ident_h = consts.tile([P, P], F16)
            ones_f = consts.tile([P, P], F32)
            ones_h1 = consts.tile([1, P], F16)
            wc_sb = consts.tile([P, ND], F16)
            wq_sb = consts.tile([P, ND], F32)
            wcq_sb = consts.tile([P, ND], F32)
            neg_shift = consts.tile([P, 1], F32)

            # params on the sync ring, identity/memsets on DVE+gpsimd --
            # all overlap the first SWDGE loads
            nc.sync.dma_start(out=wc_sb, in_=wc_d[:])
            nc.sync.dma_start(out=wq_sb, in_=wq_d[:])
            nc.sync.dma_start(out=wcq_sb, in_=wcq_d[:])

            def emit_inputs(b, chunked):
                q_sb = ins.tile([P, NJ, D + 1], F16, tag="q_sb")
                qv = q_d[b].rearrange("(t p) d -> p t d", p=P)
                if chunked:
                    nc.gpsimd.dma_start(out=q_sb[:, 0:2, 0:D], in_=qv[:, 0:2])
                    nc.gpsimd.dma_start(out=q_sb[:, 2:4, 0:D], in_=qv[:, 2:4])
                else:
                    nc.gpsimd.dma_start(out=q_sb[:, :, 0:D], in_=qv)
                nc.vector.memset(q_sb[:, :, D : D + 1], 1.0)
                c_sb = ins.tile([P, NT, D], F16, tag="c_sb")
                cv = c_d[b].rearrange("(t p) d -> p t d", p=P)
                if chunked:
                    nc.gpsimd.dma_start(out=c_sb[:, 0:KPH], in_=cv[:, 0:KPH])
                    nc.gpsimd.dma_start(out=c_sb[:, KPH:NT], in_=cv[:, KPH:NT])
                else:
                    nc.gpsimd.dma_start(out=c_sb, in_=cv)
                ov = out_d[b].rearrange("(t p) x -> p t x", p=P)
                return c_sb, q_sb, ov

            make_identity(nc, ident_h)
            nc.vector.memset(ones_f, 1.0)
            nc.vector.memset(ones_h1, 1.0)
            nc.vector.memset(neg_shift, -2.5)
            pending = [emit_inputs(0, True)]

            for b in range(bs):
                c_sb, q_sb, ov = pending.pop(0)
                # prefetch up to two batches ahead
                if b == 0:
                    for nb in (1, 2):
                        if nb < bs:
                            pending.append(emit_inputs(nb, False))
                elif b + 2 < bs:
                    pending.append(emit_inputs(b + 2, False))

                # block0 (= c) stores straight from c_sb; issued here (not at
                # prefetch time) so the sync queue never waits on a future
                # batch's load in front of this batch's stores
                nc.sync.dma_start(out=ov[:, :, 0:D], in_=c_sb)

                # ---------------- transposes (PE, fp16) ----------------
                qT = work.tile([P, ND, QL], F16, tag="qT")
                for dc in range(ND):
                    pst = ps_tr.tile([P, QL], F16, tag="tr")
                    for jc in range(NJ):
                        nc.tensor.transpose(
                            pst[:, ts(jc, P)], q_sb[:, jc, ts(dc, P)], ident_h
                        )
                    if dc == 0:
                        nc.scalar.copy(qT[:, dc], pst)
                    else:
                        nc.vector.tensor_copy(qT[:, dc], pst)

                cT = work.tile([P, ND, CL], F16, tag="cT")
                chatT = work.tile([P, ND, CL], F16, tag="chatT")
                for h in range(NH):
                    for dc in range(ND):
                        pst = ps_tr.tile([P, IH], F16, tag="tr")
                        for k in range(KPH):
                            nc.tensor.transpose(
                                pst[:, ts(k, P)],
                                c_sb[:, h * KPH + k, ts(dc, P)],
                                ident_h,
                            )
                        sl = ds(h * IH, IH)
                        nc.vector.tensor_scalar(
                            out=chatT[:, dc, sl],
                            in0=pst,
                            scalar1=wcq_sb[:, dc : dc + 1],
                            scalar2=wq_sb[:, dc : dc + 1],
                            op0=Mult,
                            op1=Add,
                        )
                        if h == 0:
                            nc.scalar.copy(cT[:, dc, sl], pst)
                        else:
                            nc.vector.tensor_copy(cT[:, dc, sl], pst)

                # ---------------- scores h0: tT + exp ----------------
                eT0 = work.tile([P, NJ, IH], F16, tag="eT0")
                eT1 = work.tile([P, NJ, IH], F16, tag="eT1")
                eT = [eT0, eT1]

                def score_chunk(h, jc):
                    pmm = ps_mm.tile([P, IH], F32, tag="tT")
                    for dc in range(ND):
                        nc.tensor.matmul(
                            pmm,
                            qT[:, dc, ts(jc, P)],
                            chatT[:, dc, ds(h * IH, IH)],
                            start=(dc == 0),
                            stop=(dc == ND - 1),
                        )
                    nc.scalar.activation(eT[h][:, jc], pmm, Exp)

                for jc in range(NJ):
                    score_chunk(0, jc)

                # ---- sc0 in column layout: 16 tiny N=1 matmuls ----
                psc = ps_tr.tile([P, NT], F32, tag="tr")
                for it in range(NT):
                    for dc in range(ND):
                        nc.tensor.matmul(
                            psc[:, it : it + 1],
                            cT[:, dc, ts(it, P)],
                            wc_sb[:, dc : dc + 1],
                            start=(dc == 0),
                            stop=(dc == ND - 1),
                        )
                e_sc0 = work.tile([P, NT], F16, tag="esc0")
                nc.scalar.activation(e_sc0, psc, Exp, bias=neg_shift[:, 0:1])

                # ---------------- scores h1 + chunk-max h0 ----------------
                for jc in range(NJ):
                    score_chunk(1, jc)
                    if jc == 1:
                        # row max over j, stage 1: max across j-chunks (DVE)
                        M1a0 = work.tile([P, 2, IH], F16, tag="m1a0")
                        nc.vector.tensor_max(
                            M1a0, eT[0][:, 0:2, :], eT[0][:, 2:4, :]
                        )
                        M1h0 = work.tile([P, IH], F16, tag="m1h0")
                        nc.vector.tensor_max(M1h0, M1a0[:, 0, :], M1a0[:, 1, :])

                # ---------------- c2q h0 + row-max transposes ----------------
                stage = stg.tile([P, NT, D2], F16, tag="stage")
                c4st = stg.tile([P, NT, D], F16, tag="c4st")
                linv = work.tile([P, NT], F32, tag="linv")
                Me16 = work.tile([P, NT], F16, tag="me")

                def mm2_tile(h, k):
                    it = h * KPH + k
                    po = ps_s.tile([P, D + 1], F32, tag="po")
                    for jc in range(NJ):
                        nc.tensor.matmul(
                            po,
                            eT[h][:, jc, ts(k, P)],
                            q_sb[:, jc],
                            start=(jc == 0),
                            stop=(jc == NJ - 1),
                        )
                    nc.vector.reciprocal(linv[:, it : it + 1], po[:, D : D + 1])
                    nc.scalar.mul(
                        stage[:, it, 0:D], po[:, 0:D], linv[:, it : it + 1]
                    )

                def m1t(h, m1h_tile):
                    # stage 2: transpose the [j,512] partial maxes, then a
                    # free-dim reduce gives m in column layout
                    ptm = ps_tr.tile([P, KPH, P], F16, tag="tr")
                    for k in range(KPH):
                        nc.tensor.transpose(
                            ptm[:, k, :], m1h_tile[:, ts(k, P)], ident_h
                        )
                    nc.vector.reduce_max(
                        Me16[:, h * KPH : (h + 1) * KPH], ptm, AxX
                    )

                mm2_tile(0, 0)
                m1t(0, M1h0)
                mm2_tile(0, 1)
                # chunk-max h1 (eT[1] complete by now)
                M1a1 = work.tile([P, 2, IH], F16, tag="m1a1")
                nc.vector.tensor_max(M1a1, eT[1][:, 0:2, :], eT[1][:, 2:4, :])
                M1h1 = work.tile([P, IH], F16, tag="m1h1")
                nc.vector.tensor_max(M1h1, M1a1[:, 0, :], M1a1[:, 1, :])
                mm2_tile(0, 2)
                m1t(1, M1h1)
                # bvec numerators: ebv = (max_j e^t) * e^(sc0-2.5)
                ebv = work.tile([P, NT], F16, tag="ebv")
                nc.vector.tensor_mul(ebv, Me16, e_sc0)
                colsum = work.tile([P, 1], F32, tag="colsum")
                nc.vector.reduce_sum(colsum, ebv, AxX)
                mm2_tile(0, 3)
                # c*c2q for h0 (one strided DVE op over 4 tiles)
                nc.vector.tensor_mul(
                    stage[:, 0:KPH, D:D2],
                    c_sb[:, 0:KPH],
                    stage[:, 0:KPH, 0:D],
                )
                nc.sync.dma_start(out=ov[:, 0:KPH, D:D3], in_=stage[:, 0:KPH])

                # ---------------- q2c chain + c2q h1 ----------------
                ps_tot = ps_q.tile([P, 1], F32, tag="q")
                nc.tensor.matmul(ps_tot, ones_f, colsum, start=True, stop=True)
                totinv = work.tile([P, 1], F32, tag="totinv")
                nc.vector.reciprocal(totinv, ps_tot)
                mm2_tile(1, 0)
                ps_q2c = ps_q.tile([1, D], F32, tag="q")
                for it in range(NT):
                    nc.tensor.matmul(
                        ps_q2c,
                        ebv[:, it : it + 1],
                        c_sb[:, it],
                        start=(it == 0),
                        stop=(it == NT - 1),
                    )
                q2c_row = work.tile([1, D], F16, tag="q2cr")
                nc.vector.tensor_scalar_mul(q2c_row, ps_q2c, totinv[0:1, 0:1])
                mm2_tile(1, 1)
                ps_q2cb = ps_q.tile([P, D], F32, tag="q")
                nc.tensor.matmul(
                    ps_q2cb, ones_h1, q2c_row, start=True, stop=True
                )
                q2c_sb = work.tile([P, D], F16, tag="q2csb")
                nc.scalar.copy(q2c_sb, ps_q2cb)
                # c*q2c into its own buffer, split across gpsimd and DVE
                for it in range(NT):
                    eng = nc.gpsimd if it % 2 == 0 else nc.vector
                    eng.tensor_mul(c4st[:, it], c_sb[:, it], q2c_sb)
                nc.sync.dma_start(out=ov[:, :, D3 : 4 * D], in_=c4st)
                mm2_tile(1, 2)
                mm2_tile(1, 3)
                nc.vector.tensor_mul(
                    stage[:, KPH:NT, D:D2],
                    c_sb[:, KPH:NT],
                    stage[:, KPH:NT, 0:D],
                )
                nc.sync.dma_start(out=ov[:, KPH:NT, D:D3], in_=stage[:, KPH:NT])

    nc.compile()
    return nc


_NC_CACHE = {}


def _get_nc(bs: int = BS):
    if bs not in _NC_CACHE:
        _NC_CACHE[bs] = build_bass(bs)
    return _NC_CACHE[bs]


def _param_maps(w_c, w_q, w_cq):
    wc_cols = np.ascontiguousarray(
        np.asarray(w_c, np.float32).reshape(ND, P).T.astype(np.float16)
    )
    wq_cols = np.ascontiguousarray(np.asarray(w_q, np.float32).reshape(ND, P).T)
    wcq_cols = np.ascontiguousarray(
        np.asarray(w_cq, np.float32).reshape(ND, P).T
    )
    return wc_cols, wq_cols, wcq_cols


def _run(c, q, w_c, w_q, w_cq, trace=False, **trace_kwargs):
    from concourse.bass_utils import run_bass_kernel_spmd

    c = np.asarray(c, np.float32)
    q = np.asarray(q, np.float32)
    wc_cols, wq_cols, wcq_cols = _param_maps(w_c, w_q, w_cq)

    nc = _get_nc(BS)
    in_maps = []
    for k in range(NCORES):
        in_maps.append(
            {
                "c": np.ascontiguousarray(c[k * BS : (k + 1) * BS]),
                "q": np.ascontiguousarray(q[k * BS : (k + 1) * BS]),
                "wc_cols": wc_cols,
                "wq_cols": wq_cols,
                "wcq_cols": wcq_cols,
            }
        )
    res = None
    last_err = None
    for attempt in range(3):
        try:
            res = run_bass_kernel_spmd(
                nc,
                in_maps,
                core_ids=list(range(NCORES)),
                trace=trace,
                **trace_kwargs,
            )
            break
        except Exception as e:  # transient device wedges clear on retry
            last_err = e
            if "UNRECOVERABLE" not in str(e) and "UNAVAILABLE" not in str(e):
                raise
    if res is None:
        raise last_err
    out = np.concatenate([res.results[k]["out"] for k in range(NCORES)], axis=0)
    return np.ascontiguousarray(out.astype(np.float32)), res


def kernel(c, q, w_c, b_c, w_q, b_q, w_cq, b_cq):
    # b_c/b_q/b_cq provably cancel in both softmaxes; output doesn't use them.
    out, _ = _run(c, q, w_c, w_q, w_cq)
    return out


# revision 10
# speedup vs baseline: 1.6056x; 1.2507x over previous
"""BiDAF attention layer on 8 Trainium2 NeuronCores (Bass/Tile), v2.

Math (per batch b):
  t[i,j]  = sum_d (c[i,d]*w_cq[d] + w_q[d]) * q[j,d]   (= cq + sq0[j])
  a       = softmax_j(t)            (biases b_c/b_q/b_cq cancel in softmax)
  c2q     = a @ q
  m[i]    = max_j t[i,j];  sc0[i] = c[i,:]@w_c
  bvec    = softmax_i(m + sc0)      (biases cancel here too)
  q2c     = bvec @ c
  out     = [c | c2q | c*c2q | c*q2c]

Sharding: data-parallel over batch, 4 batches per core, params replicated.

v2 changes vs the 136us baseline (all fp16-matmul, PE-bound):
  - Scores computed ONCE, only in the transposed [j,i] layout the c2q
    matmul needs. The row max over j (a partition-dim reduction there) is
    done as max_j t = log max_j e^t: chunk-max of e^T on DVE, 8 small PE
    transposes of the [j,512] partials, then a free-dim reduce_max. This
    kills the entire second score pass (24 N=512 matmuls per batch).
  - sc0 is computed directly in column layout by 16 tiny N=1 matmuls
    (lhsT = cT tile, rhs = w_c column), then folded into the bvec
    numerators as ebv = (max_j e^t) * exp(sc0 - 2.5).
  - c is loaded once, cast f32->fp16 in flight (SWDGE); no separate f32
    copy.  All PE work is fp16 (1 cycle/row).
  - The output is written as fp16 (tolerance is 2e-2; fp16 rounding is
    ~2e-4) and upcast to f32 on the host: halves the store traffic.
  - Output staged as one [P, NT, 3D] tile (c2q | c*c2q | c*q2c) so each
    half is a single store; block0 (= c) stores straight from c_sb.
  - DMA paths: loads on SWDGE (casts), block0 on the scalar HWDGE ring,
    stage stores on the sync ring (avoids head-of-line blocking between
    prefetched block0 stores and current-batch stage stores).
  - PE emission order interleaves transposes / score matmuls / c2q
    matmuls / q2c chain so the PE never idles waiting on ACT exp or DVE
    evacuations (idle gaps also drop the PE out of its 2.4GHz p-state).
"""

import sys

if "/opt/trn_rl_repo" not in sys.path:
    sys.path.insert(0, "/opt/trn_rl_repo")

import numpy as np

import concourse.bass as bass
import concourse.tile as tile
from concourse import bacc, mybir
from concourse.bass import ds, ts
from concourse.masks import make_identity

B, CL, QL, D = 32, 1024, 512, 256
NCORES = 8
BS = B // NCORES  # batches per core
P = 128
F32 = mybir.dt.float32
F16 = mybir.dt.float16

NT = CL // P  # 8 i-tiles
NJ = QL // P  # 4 j-chunks
ND = D // P   # 2 d-chunks
NH = 2        # i-halves for the [j,i]-layout score matmul
IH = CL // NH  # 512
KPH = NT // NH  # i-tiles per half

Exp = mybir.ActivationFunctionType.Exp
AxX = mybir.AxisListType.X
Mult = mybir.AluOpType.mult
Add = mybir.AluOpType.add


def build_bass(bs: int = BS):
    nc = bacc.Bacc(None)
    c_d = nc.declare_dram_parameter("c", [bs, CL, D], F32, isOutput=False)
    q_d = nc.declare_dram_parameter("q", [bs, QL, D], F32, isOutput=False)
    wc_d = nc.declare_dram_parameter("wc_cols", [P, ND], F16, isOutput=False)
    wq_d = nc.declare_dram_parameter("wq_cols", [P, ND], F32, isOutput=False)
    wcq_d = nc.declare_dram_parameter("wcq_cols", [P, ND], F32, isOutput=False)
    out_d = nc.declare_dram_parameter("out", [bs, CL, 4 * D], F16, isOutput=True)

    D2, D3 = 2 * D, 3 * D

    with tile.TileContext(nc) as tc:
        with (
            tc.tile_pool(name="consts", bufs=1) as consts,
            tc.tile_pool(name="ins", bufs=3) as ins,
            tc.tile_pool(name="work", bufs=2) as work,
            tc.tile_pool(name="stg", bufs=2) as stg,
            tc.tile_pool(name="ps_mm", bufs=3, space="PSUM") as ps_mm,
            tc.tile_pool(name="ps_tr", bufs=2, space="PSUM") as ps_tr,
            tc.tile_pool(name="ps_s", bufs=2, space="PSUM") as ps_s,
            tc.tile_pool(name="ps_q", bufs=1, space="PSUM") as ps_q,
        ):
            ident_h = consts.tile([P, P], F16)
            ones_f = consts.tile([P, P], F32)
            ones_h1 = consts.tile([1, P], F16)
            wc_sb = consts.tile([P, ND], F16)
            wq_sb = consts.tile([P, ND], F32)
            wcq_sb = consts.tile([P, ND], F32)
            neg_shift = consts.tile([P, 1], F32)

            # params on the sync ring, identity/memsets on DVE+gpsimd --
            # all overlap the first SWDGE loads
            nc.sync.dma_start(out=wc_sb, in_=wc_d[:])
            nc.sync.dma_start(out=wq_sb, in_=wq_d[:])
            nc.sync.dma_start(out=wcq_sb, in_=wcq_d[:])

            def emit_inputs(b, chunked):
                q_sb = ins.tile([P, NJ, D + 1], F16, tag="q_sb")
                qv = q_d[b].rearrange("(t p) d -> p t d", p=P)
                if chunked:
                    nc.gpsimd.dma_start(out=q_sb[:, 0:2, 0:D], in_=qv[:, 0:2])
                    nc.gpsimd.dma_start(out=q_sb[:, 2:4, 0:D], in_=qv[:, 2:4])
                else:
                    nc.gpsimd.dma_start(out=q_sb[:, :, 0:D], in_=qv)
                nc.vector.memset(q_sb[:, :, D : D + 1], 1.0)
                c_sb = ins.tile([P, NT, D], F16, tag="c_sb")
                cv = c_d[b].rearrange("(t p) d -> p t d", p=P)
                if chunked:
                    nc.gpsimd.dma_start(out=c_sb[:, 0:KPH], in_=cv[:, 0:KPH])
                    nc.gpsimd.dma_start(out=c_sb[:, KPH:NT], in_=cv[:, KPH:NT])
                else:
                    nc.gpsimd.dma_start(out=c_sb, in_=cv)
                ov = out_d[b].rearrange("(t p) x -> p t x", p=P)
                return c_sb, q_sb, ov

            make_identity(nc, ident_h)
            nc.vector.memset(ones_f, 1.0)
            nc.vector.memset(ones_h1, 1.0)
            nc.vector.memset(neg_shift, -2.5)
            pending = [emit_inputs(0, True)]

            for b in range(bs):
                c_sb, q_sb, ov = pending.pop(0)
                # prefetch up to two batches ahead
                if b == 0:
                    for nb in (1, 2):
                        if nb < bs:
                            pending.append(emit_inputs(nb, False))
                elif b + 2 < bs:
                    pending.append(emit_inputs(b + 2, False))

                # block0 (= c) stores straight from c_sb; issued here (not at
                # prefetch time) so the sync queue never waits on a future
                # batch's load in front of this batch's stores
                nc.sync.dma_start(out=ov[:, :, 0:D], in_=c_sb)

                # ---------------- transposes (PE, fp16) ----------------
                # qT: both d-chunks into one [P,1024] PSUM, single DVE evac
                qT = work.tile([P, ND, QL], F16, tag="qT")
                psq = ps_tr.tile([P, ND, QL], F16, tag="tr")
                for dc in range(ND):
                    for jc in range(NJ):
                        nc.tensor.transpose(
                            psq[:, dc, ts(jc, P)], q_sb[:, jc, ts(dc, P)], ident_h
                        )
                nc.vector.tensor_copy(qT, psq)

                # c: per d-chunk both halves into one [P,1024] PSUM;
                # chatT (affine) evac on DVE, plain cT evac on ACT
                cT = work.tile([P, ND, CL], F16, tag="cT")
                chatT = work.tile([P, ND, CL], F16, tag="chatT")
                for dc in range(ND):
                    pst = ps_tr.tile([P, CL], F16, tag="tr")
                    for it in range(NT):
                        nc.tensor.transpose(
                            pst[:, ts(it, P)], c_sb[:, it, ts(dc, P)], ident_h
                        )
                    nc.vector.tensor_scalar(
                        out=chatT[:, dc],
                        in0=pst,
                        scalar1=wcq_sb[:, dc : dc + 1],
                        scalar2=wq_sb[:, dc : dc + 1],
                        op0=Mult,
                        op1=Add,
                    )
                    nc.scalar.copy(cT[:, dc], pst)

                # ---------------- scores: tT + exp ----------------
                eT0 = work.tile([P, NJ, IH], F16, tag="eT0")
                eT1 = work.tile([P, NJ, IH], F16, tag="eT1")
                eT = [eT0, eT1]

                def score_chunk(h, jc):
                    pmm = ps_mm.tile([P, IH], F32, tag="tT")
                    for dc in range(ND):
                        nc.tensor.matmul(
                            pmm,
                            qT[:, dc, ts(jc, P)],
                            chatT[:, dc, ds(h * IH, IH)],
                            start=(dc == 0),
                            stop=(dc == ND - 1),
                        )
                    nc.scalar.activation(eT[h][:, jc], pmm, Exp)

                for jc in range(NJ):
                    score_chunk(0, jc)

                # sc0 in column layout: 16 tiny N=1 matmuls (fills the
                # exp-h0 latency window on the PE)
                psc = ps_tr.tile([P, NT], F32, tag="tr")
                for it in range(NT):
                    for dc in range(ND):
                        nc.tensor.matmul(
                            psc[:, it : it + 1],
                            cT[:, dc, ts(it, P)],
                            wc_sb[:, dc : dc + 1],
                            start=(dc == 0),
                            stop=(dc == ND - 1),
                        )
                e_sc0 = work.tile([P, NT], F16, tag="esc0")
                nc.scalar.activation(e_sc0, psc, Exp, bias=neg_shift[:, 0:1])

                for jc in range(NJ):
                    score_chunk(1, jc)
                    if jc == 1:
                        # row max over j, stage 1: max across j-chunks (DVE)
                        M1a0 = work.tile([P, 2, IH], F16, tag="m1a0")
                        nc.vector.tensor_max(
                            M1a0, eT[0][:, 0:2, :], eT[0][:, 2:4, :]
                        )
                        M1h0 = work.tile([P, IH], F16, tag="m1h0")
                        nc.vector.tensor_max(M1h0, M1a0[:, 0, :], M1a0[:, 1, :])

                # ------------ c2q + row-max + q2c, interleaved ------------
                stage = stg.tile([P, NT, D2], F16, tag="stage")
                c4st = stg.tile([P, NT, D], F16, tag="c4st")
                linv = work.tile([P, NT], F32, tag="linv")
                Me16 = work.tile([P, NT], F16, tag="me")

                def mm2_tile(h, k):
                    it = h * KPH + k
                    po = ps_s.tile([P, D + 1], F32, tag="po")
                    for jc in range(NJ):
                        nc.tensor.matmul(
                            po,
                            eT[h][:, jc, ts(k, P)],
                            q_sb[:, jc],
                            start=(jc == 0),
                            stop=(jc == NJ - 1),
                        )
                    nc.vector.reciprocal(linv[:, it : it + 1], po[:, D : D + 1])
                    nc.scalar.mul(
                        stage[:, it, 0:D], po[:, 0:D], linv[:, it : it + 1]
                    )

                def m1t(h, m1h_tile):
                    # stage 2: transpose the [j,512] partial maxes, then a
                    # free-dim reduce gives m in column layout
                    ptm = ps_tr.tile([P, KPH, P], F16, tag="tr")
                    for k in range(KPH):
                        nc.tensor.transpose(
                            ptm[:, k, :], m1h_tile[:, ts(k, P)], ident_h
                        )
                    nc.vector.reduce_max(
                        Me16[:, h * KPH : (h + 1) * KPH], ptm, AxX
                    )

                mm2_tile(0, 0)
                m1t(0, M1h0)
                mm2_tile(0, 1)
                # chunk-max h1 (eT1 complete by now), then its transposes
                M1a1 = work.tile([P, 2, IH], F16, tag="m1a1")
                nc.vector.tensor_max(M1a1, eT[1][:, 0:2, :], eT[1][:, 2:4, :])
                M1h1 = work.tile([P, IH], F16, tag="m1h1")
                nc.vector.tensor_max(M1h1, M1a1[:, 0, :], M1a1[:, 1, :])
                m1t(1, M1h1)
                # bvec numerators: ebv = (max_j e^t) * e^(sc0-2.5)
                ebv = work.tile([P, NT], F16, tag="ebv")
                nc.vector.tensor_mul(ebv, Me16, e_sc0)
                colsum = work.tile([P, 1], F32, tag="colsum")
                nc.vector.reduce_sum(colsum, ebv, AxX)
                mm2_tile(0, 2)
                ps_tot = ps_q.tile([P, 1], F32, tag="q")
                nc.tensor.matmul(ps_tot, ones_f, colsum, start=True, stop=True)
                totinv = work.tile([P, 1], F32, tag="totinv")
                nc.vector.reciprocal(totinv, ps_tot)
                mm2_tile(0, 3)
                ps_q2c = ps_q.tile([1, D], F32, tag="q")
                for it in range(NT):
                    nc.tensor.matmul(
                        ps_q2c,
                        ebv[:, it : it + 1],
                        c_sb[:, it],
                        start=(it == 0),
                        stop=(it == NT - 1),
                    )
                q2c_row = work.tile([1, D], F16, tag="q2cr")
                nc.vector.tensor_scalar_mul(q2c_row, ps_q2c, totinv[0:1, 0:1])
                # c*c2q h0 (one strided DVE op over 4 tiles), store h0
                nc.vector.tensor_mul(
                    stage[:, 0:KPH, D:D2],
                    c_sb[:, 0:KPH],
                    stage[:, 0:KPH, 0:D],
                )
                nc.sync.dma_start(out=ov[:, 0:KPH, D:D3], in_=stage[:, 0:KPH])
                ps_q2cb = ps_q.tile([P, D], F32, tag="q")
                nc.tensor.matmul(
                    ps_q2cb, ones_h1, q2c_row, start=True, stop=True
                )
                q2c_sb = work.tile([P, D], F16, tag="q2csb")
                nc.scalar.copy(q2c_sb, ps_q2cb)

                # c*q2c: gpsimd, except split with DVE on the last batch
                # (parallel finish matters only at the tail)
                def c4_mul(it):
                    eng = (
                        nc.vector
                        if (b == bs - 1 and it % 2 == 1)
                        else nc.gpsimd
                    )
                    eng.tensor_mul(c4st[:, it], c_sb[:, it], q2c_sb)

                for it in range(KPH):
                    c4_mul(it)
                nc.sync.dma_start(
                    out=ov[:, 0:KPH, D3 : 4 * D], in_=c4st[:, 0:KPH]
                )
                mm2_tile(1, 0)
                mm2_tile(1, 1)
                for it in range(KPH, NT):
                    c4_mul(it)
                nc.sync.dma_start(
                    out=ov[:, KPH:NT, D3 : 4 * D], in_=c4st[:, KPH:NT]
                )
                mm2_tile(1, 2)
                mm2_tile(1, 3)
                nc.vector.tensor_mul(
                    stage[:, KPH:NT, D:D2],
                    c_sb[:, KPH:NT],
                    stage[:, KPH:NT, 0:D],
                )
                nc.sync.dma_start(out=ov[:, KPH:NT, D:D3], in_=stage[:, KPH:NT])

    nc.compile()
    return nc


_NC_CACHE = {}


def _get_nc(bs: int = BS):
    if bs not in _NC_CACHE:
        _NC_CACHE[bs] = build_bass(bs)
    return _NC_CACHE[bs]


def _param_maps(w_c, w_q, w_cq):
    wc_cols = np.ascontiguousarray(
        np.asarray(w_c, np.float32).reshape(ND, P).T.astype(np.float16)
    )
    wq_cols = np.ascontiguousarray(np.asarray(w_q, np.float32).reshape(ND, P).T)
    wcq_cols = np.ascontiguousarray(
        np.asarray(w_cq, np.float32).reshape(ND, P).T
    )
    return wc_cols, wq_cols, wcq_cols


def _run(c, q, w_c, w_q, w_cq, trace=False, **trace_kwargs):
    from concourse.bass_utils import run_bass_kernel_spmd

    c = np.asarray(c, np.float32)
    q = np.asarray(q, np.float32)
    wc_cols, wq_cols, wcq_cols = _param_maps(w_c, w_q, w_cq)

    nc = _get_nc(BS)
    in_maps = []
    for k in range(NCORES):
        in_maps.append(
            {
                "c": np.ascontiguousarray(c[k * BS : (k + 1) * BS]),
                "q": np.ascontiguousarray(q[k * BS : (k + 1) * BS]),
                "wc_cols": wc_cols,
                "wq_cols": wq_cols,
                "wcq_cols": wcq_cols,
            }
        )
    res = None
    last_err = None
    for attempt in range(3):
        try:
            res = run_bass_kernel_spmd(
                nc,
                in_maps,
                core_ids=list(range(NCORES)),
                trace=trace,
                **trace_kwargs,
            )
            break
        except Exception as e:  # transient device wedges clear on retry
            last_err = e
            if "UNRECOVERABLE" not in str(e) and "UNAVAILABLE" not in str(e):
                raise
    if res is None:
        raise last_err
    out = np.concatenate([res.results[k]["out"] for k in range(NCORES)], axis=0)
    return np.ascontiguousarray(out.astype(np.float32)), res


def kernel(c, q, w_c, b_c, w_q, b_q, w_cq, b_cq):
    # b_c/b_q/b_cq provably cancel in both softmaxes; output doesn't use them.
    out, _ = _run(c, q, w_c, w_q, w_cq)
    return out
